# revision 54
# baseline (speedup 1.0000x reference)
"""Trainium2 Bass kernel for nn_Discriminator (down-projection + 16-step LSTM + head).

Computation (per reference):
    x: [512, 16, 10001] fp32
    xa = x[:, :, :10000] @ W_down                      # [B, T, 128]
    xc_t = concat([xa_t, xw_t], -1)                    # per step, [B, 129]
    LSTM over T=16 steps, H=512, forget bias +1:
        gates = [xc_t, h] @ W_cell + b_cell            # [B, 2048] = [i|c|f|o]
        c = c*sig(f+1) + sig(i)*tanh(c_)
        h = sig(o)*tanh(c)
    pred = h @ W_out + b_out                           # [B, 1]

Sharding: pure data-parallel over batch, 64 rows/core on 8 cores. No collectives.

Per-core design (measured ~169 us on trn2, vs ~270 us for the first working
version):
  - x is cast to bf16 and pre-transposed/tiled on the host so the PE
    contraction dim (n) is on partitions and every down-projection slab DMA
    is one contiguous 8 KB row-block per partition (128 descriptors/MB).
  - The down-projection accumulates xa^T in 4 column chunks (4 LSTM steps
    each); chunk 0 runs up front (DMA-paced), the rest are emitted inside
    the LSTM steps' PE-idle windows so the x stream and recurrence overlap.
  - LSTM gates for a step live in one PSUM tile [128, 1024]: partitions
    0:64 hold the [c~|i] gate columns, 64:128 hold [f|o], each 512-column
    bank fed by two concurrent PE column-group matmuls (col-tiling). Bank 0
    (c~, f) is sigmoided while bank 1's matmuls still stream.
  - All four gates use a single activation function: tanh is folded into
    sigmoid on the host (tanh(x) = 2*sig(2x)-1, c-gate columns pre-scaled
    by 2), the forget bias +1 is folded into b_cell, and the resulting
    affine corrections into W_h/W_out. The cell update is 4 DVE ops using
    scalar_tensor_tensor fusions; the cross-partition-half add reads one
    PSUM and one SBUF operand to satisfy DVE operand rules.
  - h is produced directly in "half-scale" form ((sig(2c)-0.5)*sig(o)) at
    partitions 64:128, PE-transposed (4x [64,128]) into one PSUM tile and
    copied once to form the next step's lhsT.
  - The next step's x-path gate matmuls are pre-accumulated into the next
    PSUM tile during the current step's activation chain, and anchored
    warm-keeper matmuls keep the PE's HAM clock gate at 2.4 GHz through
    the chain.
"""

import numpy as np
from contextlib import ExitStack

NCORES = 8
B = 512
BC = B // NCORES          # 64 batch rows per core
T = 16
BT = BC * T               # 1024
N = 10000
KT = 79                   # ceil(10000/128)
NPAD = KT * 128           # 10112
HIN = 128
H = 512
G4 = 4 * H                # 2048
DP_CHUNKS = 4             # down-projection output chunks (t-blocks)
CCOLS = BT // DP_CHUNKS   # columns per chunk (256)

# Compute dtype for matmul operands: "float32r" (full-rate fp32 PE mode) or
# "bfloat16" (halves HBM traffic for x; small accuracy cost).
MM_DTYPE = "bfloat16"

_CACHE = {}


def _np_mm_dtype():
    if MM_DTYPE == "bfloat16":
        import ml_dtypes
        return ml_dtypes.bfloat16
    return np.float32


def _build_module():
    import concourse.bass as bass  # noqa: F401
    import concourse.bacc as bacc
    import concourse.tile as tile
    import concourse.mybir as mybir

    AF = mybir.ActivationFunctionType
    ALU = mybir.AluOpType
    f32 = mybir.dt.float32
    mmdt = mybir.dt.bfloat16 if MM_DTYPE == "bfloat16" else mybir.dt.float32r

    nc = bacc.Bacc("TRN2")

    # x pre-tiled on host to [128, DP_CHUNKS, KT, CCOLS] (flattened along the
    # free dim) so every slab DMA is one contiguous row-block per partition
    xT = nc.declare_dram_parameter("xT", [128, DP_CHUNKS * KT * CCOLS], mmdt, isOutput=False)
    xw = nc.declare_dram_parameter("xw", [2, BT], mmdt, isOutput=False)
    Wd = nc.declare_dram_parameter("Wd", [128, KT * 128], mmdt, isOutput=False)
    Wxa = nc.declare_dram_parameter("Wxa", [128, G4], mmdt, isOutput=False)
    Wxwb = nc.declare_dram_parameter("Wxwb", [2, G4], mmdt, isOutput=False)
    Wh = nc.declare_dram_parameter("Wh", [128, 4 * G4], mmdt, isOutput=False)
    Wo = nc.declare_dram_parameter("Wo", [128, 4], mmdt, isOutput=False)
    bout = nc.declare_dram_parameter("bout", [BC, 1], f32, isOutput=False)
    ident = nc.declare_dram_parameter("ident", [128, BC], mmdt, isOutput=False)
    pred = nc.declare_dram_parameter("pred", [BC, 1], f32, isOutput=True)

    # slab granularity: KG k-tiles per DMA (bigger transfers, fewer issues)
    KG = 8
    NSLAB = (KT + KG - 1) // KG  # 10 (last slab holds 7 k-tiles)

    with ExitStack() as ctx:
        tc = ctx.enter_context(tile.TileContext(nc))
        singles = ctx.enter_context(tc.tile_pool(name="singles", bufs=1))
        slabs = ctx.enter_context(tc.tile_pool(name="slabs", bufs=6))
        work = ctx.enter_context(tc.tile_pool(name="work", bufs=2))
        state = ctx.enter_context(tc.tile_pool(name="state", bufs=2))
        dpp = ctx.enter_context(tc.tile_pool(name="dpp", bufs=2, space="PSUM"))
        gp = ctx.enter_context(tc.tile_pool(name="gp", bufs=2, space="PSUM"))
        tp = ctx.enter_context(tc.tile_pool(name="tp", bufs=2, space="PSUM"))

        # W_down first (the dp stream's only prerequisite), split across DMA
        # lanes so the first k-tiles land quickly
        Wd_sb = singles.tile([128, KT * 128], mmdt)
        wd_step = 10 * 128
        for o in range(0, KT * 128, wd_step):
            e = min(o + wd_step, KT * 128)
            nc.scalar.dma_start(Wd_sb[:, o:e], Wd[:, o:e])

        # xa^T, one tile per chunk so the LSTM's dependency is per-chunk
        xaT_sb = [singles.tile([128, CCOLS], mmdt, name=f"xaT{c}") for c in range(DP_CHUNKS)]

        # down-projection chunk-0 stream emitted first; LSTM weights after
        def warm_mms(n, rows=512):
            # tiny independent matmuls into a scratch PSUM tile: keep the
            # PE busy enough that the HAM clock gate stays at 2.4 GHz
            warm = tp.tile([128, rows], mybir.dt.float32, tag="tp")
            for w in range(n):
                nc.tensor.matmul(warm[:], Wd_sb[:, w * 128:(w + 1) * 128],
                                 Wd_sb[:, 0:rows], start=(w == 0), stop=(w == n - 1))

        def dp_slab(c, ps, s, warm=0):
            k0 = s * KG
            nk = min(KG, KT - k0)
            sl = slabs.tile([128, KG, CCOLS], mmdt, tag="slab")
            off = (c * KT + k0) * CCOLS
            nc.sync.dma_start(sl[:, :nk, :],
                              xT[:, off:off + nk * CCOLS].rearrange("p (t c) -> p t c", c=CCOLS))
            for j in range(nk):
                k = k0 + j
                nc.tensor.matmul(ps[:], Wd_sb[:, k * 128:(k + 1) * 128], sl[:, j, :],
                                 start=(k == 0), stop=(k == KT - 1))
            if warm:
                warm_mms(warm)

        def dp_chunk(c, ps):
            for s in range(NSLAB):
                dp_slab(c, ps, s, warm=3)
            nc.scalar.copy(xaT_sb[c][:], ps[:])

        ps0 = dpp.tile([128, 512], mybir.dt.float32, tag="dp", name="ps0")[:, :CCOLS]
        dp_chunk(0, ps0)

        # LSTM weights + small tensors (needed only once step 0 starts)
        Wxa_sb = singles.tile([128, G4], mmdt)
        nc.scalar.dma_start(Wxa_sb[:], Wxa[:])
        Wxwb_sb = singles.tile([2, G4], mmdt)
        nc.scalar.dma_start(Wxwb_sb[:], Wxwb[:])
        Wh_sb = singles.tile([128, 4 * G4], mmdt)
        nc.scalar.dma_start(Wh_sb[:], Wh[:])
        Wo_sb = singles.tile([128, 4], mmdt)
        nc.scalar.dma_start(Wo_sb[:], Wo[:])
        xw_sb = singles.tile([2, BT], mmdt)
        nc.scalar.dma_start(xw_sb[:], xw[:])
        bout_sb = singles.tile([BC, 1], f32)
        nc.scalar.dma_start(bout_sb[:], bout[:])
        id_sb = singles.tile([128, BC], mmdt)
        nc.scalar.dma_start(id_sb[:], ident[:])

        hT_prev = None
        # U tile: partitions 0:64 hold tanh(c~_t), 64:128 hold c_{t-1}
        U_cur = work.tile([128, H], mybir.dt.float32, tag="u")
        nc.vector.memset(U_cur[64:128, :], 0.0)
        g_next = None  # psum tile of the NEXT step, pre-accumulated with xa/xw

        def xaxw_mms(t, g, close=False):
            """Accumulate the h-independent gate contributions for step t
            into psum g (start=True). Emitted early so the PE does this work
            while the previous step's activation chain runs. close=True ends
            the accumulation group (used at t=0, which has no h terms)."""
            ktiles = [
                (xaT_sb[t * BC // CCOLS][:, (t * BC) % CCOLS:(t * BC) % CCOLS + BC], Wxa_sb),
                (xw_sb[:, t * BC:(t + 1) * BC], Wxwb_sb),
            ]
            for ki, (lh, rh) in enumerate(ktiles):
                st = ki == 0
                sp = close and ki == len(ktiles) - 1
                for ch in range(2):
                    for half in range(2):
                        outap = g[half * 64:(half + 1) * 64, ch * H:(ch + 1) * H]
                        rhap = rh[:, half * 2 * H + ch * H: half * 2 * H + (ch + 1) * H]
                        nc.tensor.matmul(outap, lh, rhap, start=st, stop=sp)

        def lstm_step(t, pe_fill=None, pe_fill_post=None):
            nonlocal hT_prev, U_cur, g_next
            g = g_next
            # h-dependent gate contributions (the recurrent critical path).
            # Gate columns are host-permuted to [c~ | i] on partitions 0:64
            # and [f | o] on 64:128, so PSUM bank 0 (cols 0:H) holds the
            # early-needed gates (c~, f): emit its matmuls first and sigmoid
            # it while bank 1's matmuls still stream.
            if hT_prev is not None:
                for ch in range(2):
                    for k in range(4):
                        lh = hT_prev[:, k, :]
                        rh = Wh_sb[:, k * G4:(k + 1) * G4]
                        sp = k == 3
                        for half in range(2):
                            outap = g[half * 64:(half + 1) * 64, ch * H:(ch + 1) * H]
                            rhap = rh[:, half * 2 * H + ch * H: half * 2 * H + (ch + 1) * H]
                            nc.tensor.matmul(outap, lh, rhap, start=False, stop=sp)
            # extra PE work (later dp chunks' slabs or HAM warm-keeper
            # matmuls) in the activation-chain shadow. Must be emitted BEFORE
            # the xa prefetch below: Tile's dependency tracking is trace-
            # order-based, and the next step may read an xaT chunk whose
            # copy is emitted by this fill.
            if pe_fill is not None:
                pe_fill()

            # pre-accumulate next step's h-independent parts while this
            # step's activation chain runs (PE would otherwise idle)
            if t + 1 < T:
                g_next = gp.tile([128, 2 * H], mybir.dt.float32, tag="g")
                xaxw_mms(t + 1, g_next)
            if pe_fill_post is not None:
                pe_fill_post()

            # Per-bank sigmoids (tanh folded in on the host:
            # tanh(x) = 2*sig(2x)-1 with c-gate columns pre-scaled by 2;
            # forget bias +1 folded into b_cell; W_h/W_out scaled by 2):
            #   S2a[0:64] = sig(2*c~)   S2a[64:128] = sig(f+1)
            #   S2b[0:64] = sig(i)      S2b[64:128] = sig(o)
            S2a = work.tile([128, H], mybir.dt.float32, tag="s2a")
            nc.scalar.activation(S2a[:], g[:, 0:H], AF.Sigmoid)
            S2b = work.tile([128, H], mybir.dt.float32, tag="s2b")
            nc.scalar.activation(S2b[:], g[:, H:2 * H], AF.Sigmoid)

            # U_cur = [tanh(c~) ; c_prev] stacked on partitions. The two LSTM
            # products go to different spaces so the cross-half add can read
            # one PSUM + one SBUF operand (two PSUM reads are illegal, and
            # two SBUF reads would violate the same-base-partition rule):
            #   m1 (psum, p64:128) = sig(f+1)*c_prev
            #   m2 (sbuf, p0:64)   = sig(i)*tanh(c~)
            m1 = tp.tile([128, H], mybir.dt.float32, tag="tp")
            nc.vector.tensor_mul(m1[64:128, :], S2a[64:128, :], U_cur[64:128, :])
            # m2' = (sig(2c~) - 0.5) * sig(i) = tanh(c~)*sig(i)/2
            m2 = work.tile([BC, H], mybir.dt.float32, tag="m2")
            nc.vector.scalar_tensor_tensor(m2[:], S2a[0:64, :], 0.5, S2b[0:64, :],
                                           ALU.subtract, ALU.mult)
            # c_new = 2*m2' + m1, written straight into the next step's U
            U_nxt = work.tile([128, H], mybir.dt.float32, tag="u")
            nc.vector.scalar_tensor_tensor(U_nxt[64:128, :], m2[:], 2.0, m1[64:128, :],
                                           ALU.mult, ALU.add)

            # h/2 = (sig(2*c_new) - 0.5) * sig(o)   [tanh via sigmoid again;
            # the missing *2 is folded into W_h and W_out on the host]
            scn = work.tile([128, H], mybir.dt.float32, tag="scn")
            nc.scalar.activation(scn[64:128, :], U_nxt[64:128, :], AF.Sigmoid, scale=2.0)
            h = work.tile([128, H], mmdt, tag="h")
            nc.vector.scalar_tensor_tensor(h[64:128, :], scn[64:128, :], 0.5,
                                           S2b[64:128, :],
                                           ALU.subtract, ALU.mult)

            # transpose h into the next step's lhsT via 4 PE transposes into
            # one PSUM tile, then a single copy
            hT = state.tile([128, 4, BC], mmdt, tag="hT")
            tps = tp.tile([128, 4 * BC], mmdt, tag="tp")
            for k in range(4):
                nc.tensor.transpose(tps[:, k * BC:(k + 1) * BC],
                                    h[64:128, k * 128:(k + 1) * 128], id_sb[64:128, :])
            nc.vector.tensor_copy(hT[:], tps[:].rearrange("p (k b) -> p k b", k=4))
            hT_prev = hT
            U_cur = U_nxt

        # LSTM: the remaining dp chunks' slabs are spread across the steps'
        # PE-idle windows (3 slabs per step, finishing each chunk well before
        # the steps that need it); once the queue drains, later steps get
        # warm-keeper matmuls so the PE's HAM clock gate stays at 2.4 GHz
        # through the activation chains.
        g_next = gp.tile([128, 2 * H], mybir.dt.float32, tag="g")
        xaxw_mms(0, g_next, close=True)
        slab_q = [(c, s) for c in range(1, DP_CHUNKS) for s in range(NSLAB)]
        cur_ps = [None] * DP_CHUNKS

        def dp_fill():
            if slab_q:
                for _ in range(min(3, len(slab_q))):
                    c, s = slab_q.pop(0)
                    if s == 0:
                        cur_ps[c] = dpp.tile([128, 512], mybir.dt.float32,
                                             tag="dp", name=f"dps{c}")[:, :CCOLS]
                    dp_slab(c, cur_ps[c], s)
                    if s == NSLAB - 1:
                        nc.scalar.copy(xaT_sb[c][:], cur_ps[c][:])

        def make_warm(t):
            def warm_fill():
                if not slab_q and t + 1 < T and hT_prev is not None:
                    # HAM warm-keepers, anchored on hT_prev so the scheduler
                    # cannot hoist them out of this step's PE stream slot
                    warm = dpp.tile([128, 512], mybir.dt.float32, tag="dp", name="warm")
                    for w in range(8):
                        nc.tensor.matmul(warm[0:64, :], hT_prev[:, w % 4, :],
                                         Wh_sb[:, 0:512], start=(w == 0), stop=(w == 7))
            return warm_fill

        for t in range(T):
            lstm_step(t, pe_fill=dp_fill, pe_fill_post=make_warm(t))

        # output head: pred = h_T @ W_out + b_out
        ps_p = tp.tile([BC, 1], mybir.dt.float32, tag="tp")
        for k in range(4):
            nc.tensor.matmul(ps_p[:], hT_prev[:, k, :], Wo_sb[:, k:k + 1],
                             start=(k == 0), stop=(k == 3))
        out_sb = singles.tile([BC, 1], mybir.dt.float32)
        nc.scalar.activation(out_sb[:], ps_p[:], AF.Identity, bias=bout_sb[:])
        nc.sync.dma_start(pred[:], out_sb[:])

    nc.finalize()
    return nc


def _get_module():
    key = MM_DTYPE
    if key not in _CACHE:
        _CACHE[key] = _build_module()
    return _CACHE[key]


def _prep_inputs(x, W_down, W_cell, b_cell, W_out, b_out):
    mmnp = _np_mm_dtype()
    x = np.asarray(x, dtype=np.float32)
    W_down = np.asarray(W_down, dtype=np.float32)
    W_cell = np.asarray(W_cell, dtype=np.float32)
    b_cell = np.asarray(b_cell, dtype=np.float32)
    W_out = np.asarray(W_out, dtype=np.float32)
    b_out = np.asarray(b_out, dtype=np.float32)

    # shared tensors
    Wd_pad = np.zeros((NPAD, HIN), dtype=np.float32)
    Wd_pad[:N] = W_down
    # [NPAD, 128] -> per-k-tile layout [128, KT*128] (col block k = k-tile)
    Wd_host = np.ascontiguousarray(
        Wd_pad.reshape(KT, 128, HIN).transpose(1, 0, 2).reshape(128, KT * HIN)
    ).astype(mmnp)
    # Fold the LSTM's fixed affine pieces into the weights so the device can
    # use a single sigmoid for all four gates (tanh(x) = 2*sig(2x) - 1) and
    # an (s - 0.5)*sig(o) form for h (which halves h; compensated by scaling
    # W_h and W_out by 2):
    #   - c-gate columns (512:1024) scaled by 2  -> sig computes sig(2*c~)
    #   - forget bias +1 folded into b_cell
    #   - W_h, W_out scaled by 2
    Wmod = W_cell.astype(np.float64).copy()
    b_mod = b_cell.astype(np.float64).copy()
    b_mod[1024:1536] += 1.0
    Wmod[:, 512:1024] *= 2.0
    b_mod[512:1024] *= 2.0
    # permute gate columns [i|c|f|o] -> [c~|i|f|o]: the device wants the
    # early-needed gates (c~, f) in PSUM bank 0 of each partition half
    perm = np.concatenate([np.arange(512, 1024), np.arange(0, 512),
                           np.arange(1024, 1536), np.arange(1536, 2048)])
    Wmod = Wmod[:, perm]
    b_mod = b_mod[perm]
    Wxa_host = np.ascontiguousarray(Wmod[0:HIN]).astype(mmnp)            # [128, 2048]
    Wxwb_host = np.stack([Wmod[HIN], b_mod]).astype(mmnp)                # [2, 2048]
    Wh_host = np.ascontiguousarray(
        (2.0 * Wmod[HIN + 1:]).reshape(4, 128, G4).transpose(1, 0, 2).reshape(128, 4 * G4)
    ).astype(mmnp)                                                       # [128, 4*2048]
    Wo_host = np.ascontiguousarray(2.0 * W_out.reshape(4, 128).T).astype(mmnp)  # [128, 4]
    bout_host = np.full((BC, 1), float(b_out[0]), dtype=np.float32)
    id_host = np.concatenate([np.eye(BC, dtype=np.float32)] * 2, axis=0).astype(mmnp)

    in_maps = []
    for i in range(NCORES):
        xs = x[i * BC:(i + 1) * BC]                       # [64, 16, 10001]
        # xT: [NPAD, 1024], column index = t*64 + b (t-major)
        xT_host = np.zeros((NPAD, BT), dtype=mmnp)
        xT_host[:N] = xs[:, :, :N].transpose(2, 1, 0).reshape(N, BT).astype(mmnp)
        # re-tile to [128, DP_CHUNKS, KT, CCOLS] flattened on the free dim so
        # each (chunk, k-group) slab is contiguous per partition:
        # xt4[p, c, k, j] = xT[k*128 + p, c*CCOLS + j]
        xt4 = xT_host.reshape(KT, 128, DP_CHUNKS, CCOLS).transpose(1, 2, 0, 3)
        xT_host = np.ascontiguousarray(xt4).reshape(128, DP_CHUNKS * KT * CCOLS)
        xw_host = np.empty((2, BT), dtype=mmnp)
        xw_host[0] = xs[:, :, N].T.reshape(BT).astype(mmnp)
        xw_host[1] = np.ones(BT, dtype=np.float32).astype(mmnp)
        in_maps.append({
            "xT": xT_host,
            "xw": xw_host,
            "Wd": Wd_host,
            "Wxa": Wxa_host,
            "Wxwb": Wxwb_host,
            "Wh": Wh_host,
            "Wo": Wo_host,
            "bout": bout_host,
            "ident": id_host,
        })
    return in_maps


def run(trace=False, **inputs):
    from concourse.bass_utils import run_bass_kernel_spmd

    nc = _get_module()
    in_maps = _prep_inputs(**inputs)
    res = run_bass_kernel_spmd(nc, in_maps, list(range(NCORES)), trace=trace)
    pred = np.concatenate([res.results[i]["pred"] for i in range(NCORES)], axis=0)
    return pred.astype(np.float32), res


def kernel(**inputs):
    pred, _ = run(trace=False, **inputs)
    return pred
